# revision 5
# baseline (speedup 1.0000x reference)
"""Causal self-attention (B=4, T=2048, C=1024, NH=16) on 8 TRN2 NeuronCores.

Sharding: tensor-parallel over heads x data-parallel over batch.
Core i handles batch b = i//2 and head-group g = i%2 (8 heads each).
  - c_attn column-parallel: each core computes q,k,v for its 8 heads.
  - attention: fully local per core (its heads, its batch element).
  - c_proj COLUMN-parallel: after each 512-query block, the two cores of a
    pair AllGather their y halves (bf16), then each computes the full
    1024-feature contraction for ITS 512 output columns. Host concatenates
    along the channel dim.

Device algorithm (per core), matmuls bf16 with fp32 PSUM accumulation:
  qT = wq^T @ xT, kT = wk^T @ xT   (feature-major; biases added on DVE);
    only the first 512 tokens up front -- later token spans are emitted as
    PE filler inside the exp-bound attention blocks, deadline-ordered.
  v  = x @ wv (token-major) + ones column per head
  per q-block Q, per head pair (2fc, 2fc+1):
    s^T[kchunk] = kT_h^T @ qT_h    (K=64 contraction, row-tiled pair)
      diagonal chunks (kc = 4Q+j): columns q < 128j are fully causal-masked
      and skipped in the QK matmul, exp, and AV matmul (N-restricted).
    p = exp(0.125 * s^T)  (ScalarE, bf16 out); remaining 128-wide triangle
      strip of diagonal chunks causal-zeroed on GpSimd.
    o^T[65,512] += v_aug_h^T @ p   (ones column -> row 64 = softmax denom)
    normalize: r = 1/denom (DVE approx recip), partition-broadcast r via a
      K=1 PE matmul (ones[1,64]^T @ r -> PSUM), y = o * bc (DVE). No DRAM
      bounce, no shift DMA: heads A/B land in separate 64-partition tiles.
  per q-block: {yA,yB} block -> DRAM -> AllGather(pair) -> SBUF [128,8,512];
  proj token-block (128 tokens): out[128,512] = sum_ko y_full_ko @ wp_ko+bp,
  interleaved one block behind attention in idle PE slots.
"""

import sys

if "/opt/trn_rl_repo" not in sys.path:
    sys.path.insert(0, "/opt/trn_rl_repo")

import numpy as np
import ml_dtypes

import concourse.bass as bass
import concourse.bacc as bacc
import concourse.mybir as mybir
import concourse.tile as tile
from concourse.bass import ts, ds
from concourse.bass_utils import run_bass_kernel_spmd

BF16 = ml_dtypes.bfloat16
N_CORES = 8
B, T, C = 4, 2048, 1024
NH, HS = 16, 64
H_LOC = NH // 2        # heads per core
F = H_LOC * HS         # 512 local qkv features
NFC = F // 128         # 4 feature chunks (one head pair each)
NKC = T // 128         # 16 key chunks
NQ = T // 512          # 4 query blocks
KO = C // 128          # 8 contraction chunks for the projections
REPLICA_GROUPS = [[0, 1], [2, 3], [4, 5], [6, 7]]

FP32 = mybir.dt.float32
BF = mybir.dt.bfloat16


def _build_nc():
    # Bacc (not plain Bass): its compile() pipeline runs
    # generate_event_semaphores, which splits sync waits so no instruction
    # carries more than the hardware allows (walrus rejects >1 otherwise).
    nc = bacc.Bacc(None, target_bir_lowering=False, num_devices=N_CORES)

    xT = nc.dram_tensor("xT", [C, T], BF, kind="ExternalInput")
    wq = nc.dram_tensor("wq", [C, F], BF, kind="ExternalInput")
    wk = nc.dram_tensor("wk", [C, F], BF, kind="ExternalInput")
    wv = nc.dram_tensor("wv", [C, F], BF, kind="ExternalInput")
    bq = nc.dram_tensor("bq", [F], FP32, kind="ExternalInput")
    bk = nc.dram_tensor("bk", [F], FP32, kind="ExternalInput")
    bv = nc.dram_tensor("bv", [F], FP32, kind="ExternalInput")
    wp = nc.dram_tensor("wp", [C, F], BF, kind="ExternalInput")  # full rows, my cols
    bp = nc.dram_tensor("bp", [F], FP32, kind="ExternalInput")   # my cols
    out = nc.dram_tensor("out", [T, F], FP32, kind="ExternalOutput")

    with tile.TileContext(nc) as tc:
        _body(tc, xT, wq, wk, wv, bq, bk, bv, wp, bp, out)
    nc.compile()
    return nc


def _body(tc, xT, wq, wk, wv, bq, bk, bv, wp, bp, out):
    nc = tc.nc
    import contextlib

    ctx = contextlib.ExitStack()
    with ctx:
        wpool = ctx.enter_context(tc.tile_pool(name="weights", bufs=1))
        apool = ctx.enter_context(tc.tile_pool(name="acts", bufs=1))
        ppool = ctx.enter_context(tc.tile_pool(name="ptiles", bufs=3))
        npool = ctx.enter_context(tc.tile_pool(name="norm", bufs=2))
        outp = ctx.enter_context(tc.tile_pool(name="outsb", bufs=3))
        agp = ctx.enter_context(tc.tile_pool(name="agsb", bufs=2))
        # PSUM budget (8 banks): sAB [128,1024] x3 bufs = 6, oA/oB 1 bank each = 2
        ps_s = ctx.enter_context(tc.tile_pool(name="ps_s", bufs=3, space="PSUM"))
        ps_o = ctx.enter_context(tc.tile_pool(name="ps_o", bufs=1, space="PSUM"))
        dpool = ctx.enter_context(tc.tile_pool(name="dram", bufs=1, space="DRAM"))

        # ---- stage inputs into SBUF, ordered so compute starts early ----
        bq_sb = wpool.tile([128, NFC], FP32)
        nc.sync.dma_start(out=bq_sb, in_=bq.rearrange("(fo p) -> p fo", p=128))
        bk_sb = wpool.tile([128, NFC], FP32)
        nc.sync.dma_start(out=bk_sb, in_=bk.rearrange("(fo p) -> p fo", p=128))

        wq_sb = wpool.tile([128, KO, F], BF)
        nc.sync.dma_start(out=wq_sb, in_=wq.rearrange("(ko p) f -> p ko f", p=128))
        wk_sb = wpool.tile([128, KO, F], BF)
        nc.sync.dma_start(out=wk_sb, in_=wk.rearrange("(ko p) f -> p ko f", p=128))

        x_sb = wpool.tile([128, KO, T], BF)
        x_ap = xT.rearrange("(ko p) t -> p ko t", p=128)
        # quarters: the first qk units need only tokens 0:512
        for xq in range(4):
            nc.sync.dma_start(
                out=x_sb[:, :, ts(xq, 512)], in_=x_ap[:, :, ts(xq, 512)]
            )

        wv_sb = wpool.tile([128, KO, F], BF)
        nc.sync.dma_start(out=wv_sb, in_=wv.rearrange("(ko p) f -> p ko f", p=128))
        bv_bc = wpool.tile([128, F], FP32)
        nc.sync.dma_start(
            out=bv_bc,
            in_=bass.AP(tensor=bv.ap().tensor, offset=0, ap=[[0, 128], [1, F]]),
        )
        wp_sb = wpool.tile([128, 2 * NFC, F], BF)  # full 1024 rows, rank order
        nc.sync.dma_start(out=wp_sb, in_=wp.rearrange("(ko p) n -> p ko n", p=128))
        bp_bc = wpool.tile([128, F], FP32)
        nc.sync.dma_start(
            out=bp_bc,
            in_=bass.AP(tensor=bp.ap().tensor, offset=0, ap=[[0, 128], [1, F]]),
        )

        # ---- persistent activations ----
        qT_sb = apool.tile([128, NFC, T], BF)   # q, feature-major
        kT_sb = apool.tile([128, NFC, T], BF)   # k, feature-major
        # v token-major, 66-stride per head: cols 0:64 = v, col 64 = ones
        v_sb = apool.tile([128, NKC, H_LOC, 66], BF)
        nc.vector.memset(v_sb[:, :, :, 64:65], 1.0)
        # attention out, feature-major; heads A/B in separate 64-part tiles
        yTa = apool.tile([64, NFC, T], BF)
        yTb = apool.tile([64, NFC, T], BF)
        ones_sb = wpool.tile([1, 64], BF)       # lhsT of the bcast matmul
        nc.vector.memset(ones_sb, 1.0)

        # per-Q-block AllGather staging (DRAM) + gathered SBUF tiles
        ag_in = [
            dpool.tile([2, 64, NFC, 512], BF, name=f"ag_in{q}") for q in range(NQ)
        ]
        ag_out = [
            dpool.tile([2, 2, 64, NFC, 512], BF, name=f"ag_out{q}")
            for q in range(NQ)
        ]
        ag_sb = {}

        # ---- qkv projection units (a minimal prefix runs up front; the
        # rest interleaves into the exp-bound attention phase as PE filler) --
        def qk_unit_half(w_sb, b_sb, dst, fc, tq):
            # one 512-token span of q^T or k^T for head-pair chunk fc
            ps = ps_s.tile([128, 1024], FP32, tag="sAB")
            for kc in range(KO):
                nc.tensor.matmul(
                    ps[:, 0:512],
                    lhsT=w_sb[:, kc, ts(fc, 128)],
                    rhs=x_sb[:, kc, ts(tq, 512)],
                    start=(kc == 0),
                    stop=(kc == KO - 1),
                )
            nc.vector.tensor_scalar_add(
                out=dst[:, fc, ts(tq, 512)],
                in0=ps[:, 0:512],
                scalar1=b_sb[:, fc : fc + 1],
            )

        def v_unit(tc_i):
            ps = ps_s.tile([128, 1024], FP32, tag="sAB")
            for kc in range(KO):
                nc.tensor.matmul(
                    ps[:, 0:512],
                    lhsT=x_sb[:, kc, ts(tc_i, 128)],
                    rhs=wv_sb[:, kc, :],
                    start=(kc == 0),
                    stop=(kc == KO - 1),
                )
            nc.vector.tensor_add(
                out=v_sb[:, tc_i, :, 0:64],
                in0=ps[:, 0:512].rearrange("p (h f) -> p h f", h=H_LOC),
                in1=bv_bc.rearrange("p (h f) -> p h f", h=H_LOC),
            )

        # prefix: exactly what attention block 0 needs (tokens 0:512)
        for fc in range(NFC):
            qk_unit_half(wq_sb, bq_sb, qT_sb, fc, 0)
            qk_unit_half(wk_sb, bk_sb, kT_sb, fc, 0)
        for tc_i in range(4):
            v_unit(tc_i)

        # deferred qkv units, deadline-ordered: block Q's filler produces the
        # token span that block Q+1 reads (kT/qT span tq=Q+1, v chunks).
        def span_units(tq):
            u = []
            for fc in range(NFC):
                u.append(lambda fc=fc, tq=tq: qk_unit_half(wq_sb, bq_sb, qT_sb, fc, tq))
                u.append(lambda fc=fc, tq=tq: qk_unit_half(wk_sb, bk_sb, kT_sb, fc, tq))
            return u

        filler_by_block = {
            0: span_units(1) + [lambda i=i: v_unit(i) for i in range(4, 8)],
            1: span_units(2) + [lambda i=i: v_unit(i) for i in range(8, 12)],
            2: span_units(3) + [lambda i=i: v_unit(i) for i in range(12, 16)],
        }

        # ---- phase 2+3: attention per q-block; c_proj pipelined one block
        # behind, riding the exp-bound attention phase's idle PE slots
        def attention_block(Q, interleave=None, filler=()):
            filler = list(filler)
            nkc = 4 * Q + 4  # causal: only key chunks 0 .. 4Q+3 contribute
            LAG = 2  # AV matmuls trail the QK/exp pipeline by this many chunks
            for fc in range(NFC):  # head pair (2fc, 2fc+1)
                oA = ps_o.tile([65, 512], FP32, tag="oA")
                oB = ps_o.tile([65, 512], FP32, tag="oB")
                pbuf = {}

                def emit_av(kc, oA=oA, oB=oB, nkc=nkc, fc=fc, Q=Q):
                    pAB = pbuf.pop(kc)
                    j = kc - 4 * Q
                    lo = 128 * j if j > 0 else 0  # first live query column
                    nc.tensor.matmul(
                        oA[:, lo:512],
                        lhsT=v_sb[:, kc, 2 * fc, 0:65],
                        rhs=pAB[:, lo:512],
                        start=(kc == 0),
                        stop=(kc == nkc - 1),
                    )
                    nc.tensor.matmul(
                        oB[:, lo:512],
                        lhsT=v_sb[:, kc, 2 * fc + 1, 0:65],
                        rhs=pAB[:, 512 + lo : 1024],
                        start=(kc == 0),
                        stop=(kc == nkc - 1),
                    )

                for kc in range(nkc):
                    j = kc - 4 * Q
                    lo = 128 * j if j > 0 else 0  # cols q<lo fully masked: skip
                    w = 512 - lo
                    # heads A and B share one 2-bank psum tile: A in cols
                    # 0:512 (array rows 0:64), B in 512:1024 (rows 64:128);
                    # the row-tiled pair runs concurrently on the PE.
                    sAB = ps_s.tile([128, 1024], FP32, tag="sAB")
                    nc.tensor.matmul(
                        sAB[:, lo:512],
                        lhsT=kT_sb[0:64, fc, ts(kc, 128)],
                        rhs=qT_sb[0:64, fc, ds(Q * 512 + lo, w)],
                        start=True,
                        stop=True,
                        tile_position=(0, 0),
                    )
                    nc.tensor.matmul(
                        sAB[:, 512 + lo : 1024],
                        lhsT=kT_sb[64:128, fc, ts(kc, 128)],
                        rhs=qT_sb[64:128, fc, ds(Q * 512 + lo, w)],
                        start=True,
                        stop=True,
                        tile_position=(64, 0),
                    )
                    pAB = ppool.tile([128, 1024], BF, tag="pAB", bufs=4)
                    s_v = sAB.rearrange("p (h q) -> p h q", h=2)
                    p_v = pAB.rearrange("p (h q) -> p h q", h=2)
                    nc.scalar.activation(
                        out=p_v[:, :, lo:512],
                        in_=s_v[:, :, lo:512],
                        func=mybir.ActivationFunctionType.Exp,
                        scale=0.125,
                    )
                    if j >= 0:
                        # causal triangle strip: zero exp of masked scores
                        # (k_global > q_global) in cols [lo, lo+128)
                        nc.gpsimd.affine_select(
                            out=p_v[:, :, lo : lo + 128],
                            in_=p_v[:, :, lo : lo + 128],
                            compare_op=mybir.AluOpType.is_ge,
                            fill=0.0,
                            base=0,
                            channel_multiplier=-1,
                            pattern=[[0, 2], [1, 128]],
                        )
                    pbuf[kc] = pAB
                    if kc >= LAG:
                        emit_av(kc - LAG)
                for kc in range(max(0, nkc - LAG), nkc):
                    emit_av(kc)
                # normalize: y_h = o[0:64] * (1/o[64]), all DVE + one tiny
                # PE broadcast -- no DRAM bounce, no partition-shift DMA.
                oA_sb = npool.tile([65, 512], FP32, tag="oAsb")
                oB_sb = npool.tile([65, 512], FP32, tag="oBsb")
                nc.vector.tensor_copy(out=oA_sb, in_=oA)
                nc.vector.tensor_copy(out=oB_sb, in_=oB)
                # custom-DVE reciprocal_approx_fast mishandles inputs at a
                # nonzero partition base -- stage row 64 down to partition 0
                rzA = npool.tile([1, 512], FP32, tag="rzA")
                rzB = npool.tile([1, 512], FP32, tag="rzB")
                nc.vector.tensor_copy(out=rzA, in_=oA_sb[64:65, :])
                nc.vector.tensor_copy(out=rzB, in_=oB_sb[64:65, :])
                rA = npool.tile([1, 512], FP32, tag="rA")
                rB = npool.tile([1, 512], FP32, tag="rB")
                nc.vector.reciprocal_approx_fast(out=rA, in_=rzA)
                nc.vector.reciprocal_approx_fast(out=rB, in_=rzB)
                # bf16 stage: keep the PE stream uniformly bf16 (fp32
                # matmuls interact badly with FWL -- HW hang territory)
                rAb = npool.tile([1, 512], BF, tag="rAb")
                rBb = npool.tile([1, 512], BF, tag="rBb")
                nc.vector.tensor_copy(out=rAb, in_=rA)
                nc.vector.tensor_copy(out=rBb, in_=rB)
                # partition-broadcast via K=1 matmul: bc[0:64] = ones^T @ r
                bc = ps_s.tile([128, 1024], FP32, tag="sAB")
                nc.tensor.matmul(
                    bc[0:64, 0:512], lhsT=ones_sb, rhs=rAb, start=True, stop=True
                )
                nc.tensor.matmul(
                    bc[0:64, 512:1024], lhsT=ones_sb, rhs=rBb, start=True, stop=True
                )
                nc.vector.tensor_mul(
                    out=yTa[:, fc, ts(Q, 512)], in0=oA_sb[0:64, :], in1=bc[0:64, 0:512]
                )
                nc.vector.tensor_mul(
                    out=yTb[:, fc, ts(Q, 512)],
                    in0=oB_sb[0:64, :],
                    in1=bc[0:64, 512:1024],
                )

                if interleave is not None and fc >= 2:
                    # slot one c_proj token-block of the previous q-block into
                    # the PE stream (fc>=2 so its AllGather has had time to
                    # land); token-blocks 2,3 run right after this block
                    proj_tb(interleave, fc - 2)
                # deferred qkv-projection units ride the same idle PE slots
                for _ in range(3):
                    if filler:
                        filler.pop(0)()

        def proj_tb(Q, tb):
            # out rows [trow*128, +128) for this core's 512 output columns:
            # full 1024-feature contraction over the gathered y (rank order)
            trow = Q * 4 + tb
            g_sb = ag_sb[Q]
            ps = ps_s.tile([128, 1024], FP32, tag="sAB")
            for ko in range(2 * NFC):
                nc.tensor.matmul(
                    ps[:, 0:512],
                    lhsT=g_sb[:, ko, ts(tb, 128)],
                    rhs=wp_sb[:, ko, :],
                    start=(ko == 0),
                    stop=(ko == 2 * NFC - 1),
                )
            o_sb = outp.tile([128, F], FP32, tag="osb")
            nc.vector.tensor_add(out=o_sb, in0=ps[:, 0:512], in1=bp_bc)
            nc.sync.dma_start(out=out.ap()[ds(trow * 128, 128), :], in_=o_sb)

        def ag_block(Q):
            # {yA,yB} block -> DRAM; AllGather over the pair; gathered -> SBUF
            nc.sync.dma_start(out=ag_in[Q][0], in_=yTa[:, :, ts(Q, 512)])
            nc.sync.dma_start(out=ag_in[Q][1], in_=yTb[:, :, ts(Q, 512)])
            cc = nc.gpsimd.collective_compute(
                "AllGather",
                mybir.AluOpType.bypass,
                replica_groups=REPLICA_GROUPS,
                ins=[ag_in[Q][:]],
                outs=[ag_out[Q][:]],
            )
            g_sb = agp.tile([128, 2 * NFC, 512], BF, tag="agsb")
            g_v = g_sb.rearrange("p (r f) q -> p r f q", r=2)
            # head-A halves -> partitions 0:64, head-B -> 64:128; ko = (r, f)
            nc.sync.dma_start(
                out=g_v[0:64], in_=ag_out[Q][:, 0].rearrange("r p f q -> p r f q")
            )
            nc.sync.dma_start(
                out=g_v[64:128], in_=ag_out[Q][:, 1].rearrange("r p f q -> p r f q")
            )
            ag_sb[Q] = g_sb
            return cc

        # software pipeline: block Q's AllGather launches as soon as its
        # attention finishes; its c_proj matmuls interleave into block Q+1's
        # exp-bound attention phase plus the slot right after.
        for Q in range(NQ):
            attention_block(
                Q,
                interleave=Q - 1 if Q > 0 else None,
                filler=filler_by_block.get(Q, ()),
            )
            ag_block(Q)
            if Q > 0:
                proj_tb(Q - 1, 2)
                proj_tb(Q - 1, 3)
        for tb in range(4):
            proj_tb(NQ - 1, tb)


_NC_CACHE = None


def _get_nc():
    global _NC_CACHE
    if _NC_CACHE is None:
        _NC_CACHE = _build_nc()
    return _NC_CACHE


def kernel(x, w_attn, b_attn, w_proj, b_proj):
    x = np.asarray(x)
    w_attn = np.asarray(w_attn)
    b_attn = np.asarray(b_attn)
    w_proj = np.asarray(w_proj)
    b_proj = np.asarray(b_proj)

    nc = _get_nc()

    in_maps = []
    for i in range(N_CORES):
        b, g = i // 2, i % 2
        in_maps.append(
            {
                "xT": np.ascontiguousarray(x[b].T).astype(BF16),
                "wq": np.ascontiguousarray(w_attn[:, g * F : (g + 1) * F]).astype(BF16),
                "wk": np.ascontiguousarray(
                    w_attn[:, C + g * F : C + (g + 1) * F]
                ).astype(BF16),
                "wv": np.ascontiguousarray(
                    w_attn[:, 2 * C + g * F : 2 * C + (g + 1) * F]
                ).astype(BF16),
                "bq": np.ascontiguousarray(b_attn[g * F : (g + 1) * F]).astype(
                    np.float32
                ),
                "bk": np.ascontiguousarray(b_attn[C + g * F : C + (g + 1) * F]).astype(
                    np.float32
                ),
                "bv": np.ascontiguousarray(
                    b_attn[2 * C + g * F : 2 * C + (g + 1) * F]
                ).astype(np.float32),
                # c_proj column-parallel: full rows, this core's columns
                "wp": np.ascontiguousarray(w_proj[:, g * F : (g + 1) * F]).astype(BF16),
                "bp": np.ascontiguousarray(b_proj[g * F : (g + 1) * F]).astype(
                    np.float32
                ),
            }
        )

    global _last_in_maps
    _last_in_maps = in_maps  # stashed for external profiling harnesses
    res = run_bass_kernel_spmd(nc, in_maps, core_ids=list(range(N_CORES)))

    # Core (2b+g) holds output columns [g*512,(g+1)*512) for batch b.
    out = np.empty((B, T, C), dtype=np.float32)
    for b in range(B):
        out[b, :, 0:F] = res.results[2 * b]["out"]
        out[b, :, F:C] = res.results[2 * b + 1]["out"]
    return out


# revision 8
# speedup vs baseline: 1.4774x; 1.4774x over previous
"""Causal self-attention (B=4, T=2048, C=1024, NH=16) on 8 TRN2 NeuronCores.

Sharding: tensor-parallel over heads x data-parallel over batch.
Core i handles batch b = i//2 and head-group g = i%2 (8 heads each).
  - c_attn column-parallel: each core computes q,k,v for its 8 heads.
  - attention: fully local per core (its heads, its batch element).
  - c_proj COLUMN-parallel: after each 512-query block, the two cores of a
    pair AllGather their y halves (bf16), then each computes the full
    1024-feature contraction for ITS 512 output columns. Host concatenates
    along the channel dim.

Device algorithm (per core), matmuls bf16 with fp32 PSUM accumulation:
  qT = wq^T @ xT, kT = wk^T @ xT   (feature-major; biases added on DVE);
    only the first 512 tokens up front -- later token spans are emitted as
    PE filler inside the exp-bound attention blocks, deadline-ordered.
  v  = x @ wv (token-major) + ones column per head
  per q-block Q, per head pair (2fc, 2fc+1):
    s^T[kchunk] = kT_h^T @ qT_h    (K=64 contraction, row-tiled pair)
      diagonal chunks (kc = 4Q+j): columns q < 128j are fully causal-masked
      and skipped in the QK matmul, exp, and AV matmul (N-restricted).
    p = exp(0.125 * s^T)  (ScalarE, bf16 out); remaining 128-wide triangle
      strip of diagonal chunks causal-zeroed on GpSimd.
    o^T[65,512] += v_aug_h^T @ p   (ones column -> row 64 = softmax denom)
    normalize: r = 1/denom (DVE approx recip), partition-broadcast r via a
      K=1 PE matmul (ones[1,64]^T @ r -> PSUM), y = o * bc (DVE). No DRAM
      bounce, no shift DMA: heads A/B land in separate 64-partition tiles.
  per q-block: {yA,yB} block -> DRAM -> AllGather(pair) -> SBUF [128,8,512];
  proj token-block (128 tokens): out[128,512] = sum_ko y_full_ko @ wp_ko+bp,
  interleaved one block behind attention in idle PE slots.
"""

import sys

if "/opt/trn_rl_repo" not in sys.path:
    sys.path.insert(0, "/opt/trn_rl_repo")

import numpy as np
import ml_dtypes

import concourse.bass as bass
import concourse.bacc as bacc
import concourse.mybir as mybir
import concourse.tile as tile
from concourse.bass import ts, ds
from concourse.bass_utils import run_bass_kernel_spmd

BF16 = ml_dtypes.bfloat16
N_CORES = 8
B, T, C = 4, 2048, 1024
NH, HS = 16, 64
H_LOC = NH // 2        # heads per core
F = H_LOC * HS         # 512 local qkv features
NFC = F // 128         # 4 feature chunks (one head pair each)
NKC = T // 128         # 16 key chunks
NQ = T // 512          # 4 query blocks
KO = C // 128          # 8 contraction chunks for the projections
REPLICA_GROUPS = [[0, 1], [2, 3], [4, 5], [6, 7]]

FP32 = mybir.dt.float32
BF = mybir.dt.bfloat16


def _build_nc():
    # Bacc (not plain Bass): its compile() pipeline runs
    # generate_event_semaphores, which splits sync waits so no instruction
    # carries more than the hardware allows (walrus rejects >1 otherwise).
    nc = bacc.Bacc(None, target_bir_lowering=False, num_devices=N_CORES)

    xT = nc.dram_tensor("xT", [C, T], BF, kind="ExternalInput")
    wq = nc.dram_tensor("wq", [C, F], BF, kind="ExternalInput")
    wk = nc.dram_tensor("wk", [C, F], BF, kind="ExternalInput")
    wv = nc.dram_tensor("wv", [C, F], BF, kind="ExternalInput")
    bq = nc.dram_tensor("bq", [F], FP32, kind="ExternalInput")
    bk = nc.dram_tensor("bk", [F], FP32, kind="ExternalInput")
    bv = nc.dram_tensor("bv", [F], FP32, kind="ExternalInput")
    wp = nc.dram_tensor("wp", [C, F], BF, kind="ExternalInput")  # full rows, my cols
    bp = nc.dram_tensor("bp", [F], FP32, kind="ExternalInput")   # my cols
    out = nc.dram_tensor("out", [T, F], FP32, kind="ExternalOutput")

    with tile.TileContext(nc) as tc:
        _body(tc, xT, wq, wk, wv, bq, bk, bv, wp, bp, out)
    nc.compile()
    return nc


def _body(tc, xT, wq, wk, wv, bq, bk, bv, wp, bp, out):
    nc = tc.nc
    import contextlib

    ctx = contextlib.ExitStack()
    with ctx:
        wpool = ctx.enter_context(tc.tile_pool(name="weights", bufs=1))
        apool = ctx.enter_context(tc.tile_pool(name="acts", bufs=1))
        ppool = ctx.enter_context(tc.tile_pool(name="ptiles", bufs=3))
        npool = ctx.enter_context(tc.tile_pool(name="norm", bufs=2))
        outp = ctx.enter_context(tc.tile_pool(name="outsb", bufs=3))
        agp = ctx.enter_context(tc.tile_pool(name="agsb", bufs=2))
        # PSUM budget (8 banks): sAB [128,1024] x3 bufs = 6, oA/oB 1 bank each = 2
        ps_s = ctx.enter_context(tc.tile_pool(name="ps_s", bufs=3, space="PSUM"))
        ps_o = ctx.enter_context(tc.tile_pool(name="ps_o", bufs=1, space="PSUM"))
        dpool = ctx.enter_context(tc.tile_pool(name="dram", bufs=1, space="DRAM"))

        # ---- stage inputs into SBUF, ordered so compute starts early ----
        bq_sb = wpool.tile([128, NFC], FP32)
        nc.sync.dma_start(out=bq_sb, in_=bq.rearrange("(fo p) -> p fo", p=128))
        bk_sb = wpool.tile([128, NFC], FP32)
        nc.sync.dma_start(out=bk_sb, in_=bk.rearrange("(fo p) -> p fo", p=128))

        wq_sb = wpool.tile([128, KO, F], BF)
        nc.sync.dma_start(out=wq_sb, in_=wq.rearrange("(ko p) f -> p ko f", p=128))
        wk_sb = wpool.tile([128, KO, F], BF)
        nc.sync.dma_start(out=wk_sb, in_=wk.rearrange("(ko p) f -> p ko f", p=128))

        x_sb = wpool.tile([128, KO, T], BF)
        x_ap = xT.rearrange("(ko p) t -> p ko t", p=128)
        # quarters: the first qk units need only tokens 0:512
        for xq in range(4):
            nc.sync.dma_start(
                out=x_sb[:, :, ts(xq, 512)], in_=x_ap[:, :, ts(xq, 512)]
            )

        wv_sb = wpool.tile([128, KO, F], BF)
        nc.sync.dma_start(out=wv_sb, in_=wv.rearrange("(ko p) f -> p ko f", p=128))
        bv_bc = wpool.tile([128, F], FP32)
        nc.sync.dma_start(
            out=bv_bc,
            in_=bass.AP(tensor=bv.ap().tensor, offset=0, ap=[[0, 128], [1, F]]),
        )
        wp_sb = wpool.tile([128, 2 * NFC, F], BF)  # full 1024 rows, rank order
        nc.sync.dma_start(out=wp_sb, in_=wp.rearrange("(ko p) n -> p ko n", p=128))
        bp_bc = wpool.tile([128, F], FP32)
        nc.sync.dma_start(
            out=bp_bc,
            in_=bass.AP(tensor=bp.ap().tensor, offset=0, ap=[[0, 128], [1, F]]),
        )

        # ---- persistent activations ----
        qT_sb = apool.tile([128, NFC, T], BF)   # q, feature-major
        kT_sb = apool.tile([128, NFC, T], BF)   # k, feature-major
        # v token-major, 66-stride per head: cols 0:64 = v, col 64 = ones
        v_sb = apool.tile([128, NKC, H_LOC, 66], BF)
        nc.vector.memset(v_sb[:, :, :, 64:65], 1.0)
        # attention out, feature-major; heads A/B in separate 64-part tiles
        yTa = apool.tile([64, NFC, T], BF)
        yTb = apool.tile([64, NFC, T], BF)
        ones_sb = wpool.tile([1, 64], BF)       # lhsT of the bcast matmul
        nc.vector.memset(ones_sb, 1.0)

        # per-Q-block AllGather staging (DRAM) + gathered SBUF tiles
        ag_in = [
            dpool.tile([2, 64, NFC, 512], BF, name=f"ag_in{q}") for q in range(NQ)
        ]
        ag_out = [
            dpool.tile([2, 2, 64, NFC, 512], BF, name=f"ag_out{q}")
            for q in range(NQ)
        ]
        ag_sb = {}

        # ---- qkv projection units (a minimal prefix runs up front; the
        # rest interleaves into the exp-bound attention phase as PE filler) --
        def qk_unit_half(w_sb, b_sb, dst, fc, tq):
            # one 512-token span of q^T or k^T for head-pair chunk fc
            ps = ps_s.tile([128, 1024], FP32, tag="sAB")
            for kc in range(KO):
                nc.tensor.matmul(
                    ps[:, 0:512],
                    lhsT=w_sb[:, kc, ts(fc, 128)],
                    rhs=x_sb[:, kc, ts(tq, 512)],
                    start=(kc == 0),
                    stop=(kc == KO - 1),
                )
            nc.vector.tensor_scalar_add(
                out=dst[:, fc, ts(tq, 512)],
                in0=ps[:, 0:512],
                scalar1=b_sb[:, fc : fc + 1],
            )

        def v_unit(tc_i):
            ps = ps_s.tile([128, 1024], FP32, tag="sAB")
            for kc in range(KO):
                nc.tensor.matmul(
                    ps[:, 0:512],
                    lhsT=x_sb[:, kc, ts(tc_i, 128)],
                    rhs=wv_sb[:, kc, :],
                    start=(kc == 0),
                    stop=(kc == KO - 1),
                )
            nc.vector.tensor_add(
                out=v_sb[:, tc_i, :, 0:64],
                in0=ps[:, 0:512].rearrange("p (h f) -> p h f", h=H_LOC),
                in1=bv_bc.rearrange("p (h f) -> p h f", h=H_LOC),
            )

        # prefix: exactly what attention block 0 needs (tokens 0:512)
        for fc in range(NFC):
            qk_unit_half(wq_sb, bq_sb, qT_sb, fc, 0)
            qk_unit_half(wk_sb, bk_sb, kT_sb, fc, 0)
        for tc_i in range(4):
            v_unit(tc_i)

        # deferred qkv units, deadline-ordered: block Q's filler produces the
        # token span that block Q+1 reads (kT/qT span tq=Q+1, v chunks).
        def span_units(tq):
            u = []
            for fc in range(NFC):
                u.append(lambda fc=fc, tq=tq: qk_unit_half(wq_sb, bq_sb, qT_sb, fc, tq))
                u.append(lambda fc=fc, tq=tq: qk_unit_half(wk_sb, bk_sb, kT_sb, fc, tq))
            return u

        filler_by_block = {
            0: span_units(1) + [lambda i=i: v_unit(i) for i in range(4, 8)],
            1: span_units(2) + [lambda i=i: v_unit(i) for i in range(8, 12)],
            2: span_units(3) + [lambda i=i: v_unit(i) for i in range(12, 16)],
        }

        # ---- phase 2+3: attention per q-block; c_proj pipelined one block
        # behind, riding the exp-bound attention phase's idle PE slots
        def attention_block(Q, interleave=None, filler=()):
            filler = list(filler)
            nkc = 4 * Q + 4  # causal: only key chunks 0 .. 4Q+3 contribute
            LAG = 2  # AV matmuls trail the QK/exp pipeline by this many chunks
            stage2_pending = [None]  # deferred normalize finish (see below)
            for fc in range(NFC):  # head pair (2fc, 2fc+1)
                oA = ps_o.tile([65, 512], FP32, tag="oA")
                oB = ps_o.tile([65, 512], FP32, tag="oB")
                pbuf = {}

                def emit_av(kc, oA=oA, oB=oB, nkc=nkc, fc=fc, Q=Q):
                    pAB = pbuf.pop(kc)
                    j = kc - 4 * Q
                    lo = 128 * j if j > 0 else 0  # first live query column
                    nc.tensor.matmul(
                        oA[:, lo:512],
                        lhsT=v_sb[:, kc, 2 * fc, 0:65],
                        rhs=pAB[:, lo:512],
                        start=(kc == 0),
                        stop=(kc == nkc - 1),
                    )
                    nc.tensor.matmul(
                        oB[:, lo:512],
                        lhsT=v_sb[:, kc, 2 * fc + 1, 0:65],
                        rhs=pAB[:, 512 + lo : 1024],
                        start=(kc == 0),
                        stop=(kc == nkc - 1),
                    )

                for kc in range(nkc):
                    j = kc - 4 * Q
                    lo = 128 * j if j > 0 else 0  # cols q<lo fully masked: skip
                    w = 512 - lo
                    # heads A and B share one 2-bank psum tile: A in cols
                    # 0:512 (array rows 0:64), B in 512:1024 (rows 64:128);
                    # the row-tiled pair runs concurrently on the PE.
                    sAB = ps_s.tile([128, 1024], FP32, tag="sAB")
                    nc.tensor.matmul(
                        sAB[:, lo:512],
                        lhsT=kT_sb[0:64, fc, ts(kc, 128)],
                        rhs=qT_sb[0:64, fc, ds(Q * 512 + lo, w)],
                        start=True,
                        stop=True,
                        tile_position=(0, 0),
                    )
                    nc.tensor.matmul(
                        sAB[:, 512 + lo : 1024],
                        lhsT=kT_sb[64:128, fc, ts(kc, 128)],
                        rhs=qT_sb[64:128, fc, ds(Q * 512 + lo, w)],
                        start=True,
                        stop=True,
                        tile_position=(64, 0),
                    )
                    pAB = ppool.tile([128, 1024], BF, tag="pAB", bufs=4)
                    s_v = sAB.rearrange("p (h q) -> p h q", h=2)
                    p_v = pAB.rearrange("p (h q) -> p h q", h=2)
                    nc.scalar.activation(
                        out=p_v[:, :, lo:512],
                        in_=s_v[:, :, lo:512],
                        func=mybir.ActivationFunctionType.Exp,
                        scale=0.125,
                    )
                    if j >= 0:
                        # causal triangle strip: zero exp of masked scores
                        # (k_global > q_global) in cols [lo, lo+128)
                        nc.gpsimd.affine_select(
                            out=p_v[:, :, lo : lo + 128],
                            in_=p_v[:, :, lo : lo + 128],
                            compare_op=mybir.AluOpType.is_ge,
                            fill=0.0,
                            base=0,
                            channel_multiplier=-1,
                            pattern=[[0, 2], [1, 128]],
                        )
                    pbuf[kc] = pAB
                    if kc >= LAG:
                        emit_av(kc - LAG)
                for kc in range(max(0, nkc - LAG), nkc):
                    emit_av(kc)
                # normalize: y_h = o[0:64] * (1/o[64]). Stage 1 (DVE chain:
                # copy out of PSUM, reciprocal, bf16 cast) runs now; stage 2
                # (K=1 PE broadcast matmul + multiplies) is deferred until
                # after the NEXT head pair's chunks so the in-order PE queue
                # never waits on the DVE chain.
                oA_sb = npool.tile([65, 512], FP32, tag="oAsb")
                oB_sb = npool.tile([65, 512], FP32, tag="oBsb")
                nc.vector.tensor_copy(out=oA_sb, in_=oA)
                nc.vector.tensor_copy(out=oB_sb, in_=oB)
                # custom-DVE reciprocal_approx_fast mishandles inputs at a
                # nonzero partition base -- stage row 64 down to partition 0
                rzA = npool.tile([1, 512], FP32, tag="rzA")
                rzB = npool.tile([1, 512], FP32, tag="rzB")
                nc.vector.tensor_copy(out=rzA, in_=oA_sb[64:65, :])
                nc.vector.tensor_copy(out=rzB, in_=oB_sb[64:65, :])
                rA = npool.tile([1, 512], FP32, tag="rA")
                rB = npool.tile([1, 512], FP32, tag="rB")
                nc.vector.reciprocal_approx_fast(out=rA, in_=rzA)
                nc.vector.reciprocal_approx_fast(out=rB, in_=rzB)
                # bf16 stage: keep the PE stream uniformly bf16 (fp32
                # matmuls interact badly with FWL -- HW hang territory)
                rAb = npool.tile([1, 512], BF, tag="rAb")
                rBb = npool.tile([1, 512], BF, tag="rBb")
                nc.vector.tensor_copy(out=rAb, in_=rA)
                nc.vector.tensor_copy(out=rBb, in_=rB)

                def stage2(fc=fc, oA_sb=oA_sb, oB_sb=oB_sb, rAb=rAb, rBb=rBb):
                    # partition-broadcast via K=1 matmul: bc[0:64] = ones^T @ r
                    bc = ps_s.tile([128, 1024], FP32, tag="sAB")
                    nc.tensor.matmul(
                        bc[0:64, 0:512], lhsT=ones_sb, rhs=rAb, start=True, stop=True
                    )
                    nc.tensor.matmul(
                        bc[0:64, 512:1024], lhsT=ones_sb, rhs=rBb, start=True, stop=True
                    )
                    nc.vector.tensor_mul(
                        out=yTa[:, fc, ts(Q, 512)],
                        in0=oA_sb[0:64, :],
                        in1=bc[0:64, 0:512],
                    )
                    nc.vector.tensor_mul(
                        out=yTb[:, fc, ts(Q, 512)],
                        in0=oB_sb[0:64, :],
                        in1=bc[0:64, 512:1024],
                    )

                prev = stage2_pending[0]
                stage2_pending[0] = stage2
                if prev is not None:
                    prev()

                if interleave is not None and fc >= 2:
                    # slot one c_proj token-block of the previous q-block into
                    # the PE stream (fc>=2 so its AllGather has had time to
                    # land); token-blocks 2,3 run right after this block
                    proj_tb(interleave, fc - 2)
                # deferred qkv-projection units ride the same idle PE slots
                for _ in range(3):
                    if filler:
                        filler.pop(0)()
            stage2_pending[0]()  # last head pair's normalize finish

        def proj_tb(Q, tb):
            # out rows [trow*128, +128) for this core's 512 output columns:
            # full 1024-feature contraction over the gathered y (rank order)
            trow = Q * 4 + tb
            g_sb = ag_sb[Q]
            ps = ps_s.tile([128, 1024], FP32, tag="sAB")
            for ko in range(2 * NFC):
                nc.tensor.matmul(
                    ps[:, 0:512],
                    lhsT=g_sb[:, ko, ts(tb, 128)],
                    rhs=wp_sb[:, ko, :],
                    start=(ko == 0),
                    stop=(ko == 2 * NFC - 1),
                )
            o_sb = outp.tile([128, F], FP32, tag="osb")
            nc.vector.tensor_add(out=o_sb, in0=ps[:, 0:512], in1=bp_bc)
            nc.sync.dma_start(out=out.ap()[ds(trow * 128, 128), :], in_=o_sb)

        def ag_block(Q):
            # {yA,yB} block -> DRAM; AllGather over the pair; gathered -> SBUF
            nc.sync.dma_start(out=ag_in[Q][0], in_=yTa[:, :, ts(Q, 512)])
            nc.sync.dma_start(out=ag_in[Q][1], in_=yTb[:, :, ts(Q, 512)])
            cc = nc.gpsimd.collective_compute(
                "AllGather",
                mybir.AluOpType.bypass,
                replica_groups=REPLICA_GROUPS,
                ins=[ag_in[Q][:]],
                outs=[ag_out[Q][:]],
            )
            g_sb = agp.tile([128, 2 * NFC, 512], BF, tag="agsb")
            g_v = g_sb.rearrange("p (r f) q -> p r f q", r=2)
            # head-A halves -> partitions 0:64, head-B -> 64:128; ko = (r, f)
            nc.sync.dma_start(
                out=g_v[0:64], in_=ag_out[Q][:, 0].rearrange("r p f q -> p r f q")
            )
            nc.sync.dma_start(
                out=g_v[64:128], in_=ag_out[Q][:, 1].rearrange("r p f q -> p r f q")
            )
            ag_sb[Q] = g_sb
            return cc

        # software pipeline: block Q's AllGather launches as soon as its
        # attention finishes; its c_proj matmuls interleave into block Q+1's
        # exp-bound attention phase plus the slot right after.
        for Q in range(NQ):
            attention_block(
                Q,
                interleave=Q - 1 if Q > 0 else None,
                filler=filler_by_block.get(Q, ()),
            )
            ag_block(Q)
            if Q > 0:
                proj_tb(Q - 1, 2)
                proj_tb(Q - 1, 3)
        for tb in range(4):
            proj_tb(NQ - 1, tb)


_NC_CACHE = None


def _get_nc():
    global _NC_CACHE
    if _NC_CACHE is None:
        _NC_CACHE = _build_nc()
    return _NC_CACHE


def kernel(x, w_attn, b_attn, w_proj, b_proj):
    x = np.asarray(x)
    w_attn = np.asarray(w_attn)
    b_attn = np.asarray(b_attn)
    w_proj = np.asarray(w_proj)
    b_proj = np.asarray(b_proj)

    nc = _get_nc()

    in_maps = []
    for i in range(N_CORES):
        b, g = i // 2, i % 2
        in_maps.append(
            {
                "xT": np.ascontiguousarray(x[b].T).astype(BF16),
                "wq": np.ascontiguousarray(w_attn[:, g * F : (g + 1) * F]).astype(BF16),
                "wk": np.ascontiguousarray(
                    w_attn[:, C + g * F : C + (g + 1) * F]
                ).astype(BF16),
                "wv": np.ascontiguousarray(
                    w_attn[:, 2 * C + g * F : 2 * C + (g + 1) * F]
                ).astype(BF16),
                "bq": np.ascontiguousarray(b_attn[g * F : (g + 1) * F]).astype(
                    np.float32
                ),
                "bk": np.ascontiguousarray(b_attn[C + g * F : C + (g + 1) * F]).astype(
                    np.float32
                ),
                "bv": np.ascontiguousarray(
                    b_attn[2 * C + g * F : 2 * C + (g + 1) * F]
                ).astype(np.float32),
                # c_proj column-parallel: full rows, this core's columns
                "wp": np.ascontiguousarray(w_proj[:, g * F : (g + 1) * F]).astype(BF16),
                "bp": np.ascontiguousarray(b_proj[g * F : (g + 1) * F]).astype(
                    np.float32
                ),
            }
        )

    global _last_in_maps
    _last_in_maps = in_maps  # stashed for external profiling harnesses
    res = run_bass_kernel_spmd(nc, in_maps, core_ids=list(range(N_CORES)))

    # Core (2b+g) holds output columns [g*512,(g+1)*512) for batch b.
    out = np.empty((B, T, C), dtype=np.float32)
    for b in range(B):
        out[b, :, 0:F] = res.results[2 * b]["out"]
        out[b, :, F:C] = res.results[2 * b + 1]["out"]
    return out
